# revision 13
# baseline (speedup 1.0000x reference)
"""DDSP decoder (GRU + MLP + heads) as a Bass/Tile kernel on 8 trn2 cores.

Algorithm
---------
T=32768 timesteps split over 8 cores (TC=4096 each). The sequential GRU is
solved with block-Picard iteration: each core's chunk forms NB=128 blocks of
B=32 steps. A sweep runs B sequential *batched* steps (one state column per
block), so the recurrent matvec becomes a [3H x H] @ [H x NB] GEMM. Block b
starts from block b-1's final state of the PREVIOUS sweep; boundary error
contracts by ~L^B per sweep (measured: 2.3e-3 after 2 sweeps, 2.4e-6 after
3). Cross-core boundaries travel through one small AllGather per sweep; the
rank-dependent "previous core" pick uses host-supplied one-hot masks.

Everything runs in transposed [feature, time] layout. Time order inside a
core is "s-order": s = j*NB + b  <->  t = b*B + j (j=step, b=block). The
host pre-permutes f0/loudness into s-order; the device writes outputs back
to natural order with strided DMA.

Matmuls use float32r (1 cycle/row on the PE at these widths, ~1e-4 rel
err); gate math in fp32 on DVE/ACT/GPSIMD.
"""
import math
import os
import sys
import types
from contextlib import ExitStack

import numpy as np

import concourse.bass as bass
import concourse.tile as tile
from concourse import bacc, mybir
from concourse.bass_utils import run_bass_kernel_spmd

dt = mybir.dt
AF = mybir.ActivationFunctionType
ALU = mybir.AluOpType
AX = mybir.AxisListType

T = 32768
H = 1024
H3 = 3 * H
N_HARM = 100
N_NOISE = 65
N_MLP = 3
LN_EPS = 1e-6
LOG_EXP = math.log(10.0)
NCORES = 8
TC = T // NCORES
P = 128
KH = H // P                # 8
M3 = H3 // P               # 24
B = 32                     # block length = steps per sweep
NB = TC // B               # 128 = batched width
NSWEEP = int(os.environ.get("DDSP_SWEEPS", "3"))
NH = 1 + N_HARM + N_NOISE  # 166
CHUNK = 512
NCH = TC // CHUNK          # 8
F32R = dt.float32r


def _install_ntff_hook():
    try:
        import antenv
        if "antenv.axon_hooks" in sys.modules:
            return
        mod = types.ModuleType("antenv.axon_hooks")
        state = {"hook": None}
        mod.set_axon_ntff_profile_hook = lambda h: state.__setitem__("hook", h)
        mod.get_axon_ntff_profile_hook = lambda: state["hook"]
        sys.modules["antenv.axon_hooks"] = mod
        antenv.axon_hooks = mod
        from trn_agent_boot.trn_boot import _ntff_profile_via_ctypes
        mod.set_axon_ntff_profile_hook(
            _ntff_profile_via_ctypes("/opt/axon/libaxon_pjrt.so"))
    except Exception:
        pass


def build():
    nc = bacc.Bacc("TRN2", target_bir_lowering=False, debug=False,
                   num_devices=NCORES)

    f0_in = nc.dram_tensor("f0c", [TC], dt.float32, kind="ExternalInput").ap()
    lo_in = nc.dram_tensor("loudc", [TC], dt.float32, kind="ExternalInput").ap()
    win_in = nc.dram_tensor("W_in", [2, H], dt.float32, kind="ExternalInput").ap()
    bin_in = nc.dram_tensor("b_in", [H], dt.float32, kind="ExternalInput").ap()
    wi_in = nc.dram_tensor("Wi", [H, H3], dt.float32, kind="ExternalInput").ap()
    wh_in = nc.dram_tensor("Wh", [H, H3], dt.float32, kind="ExternalInput").ap()
    bh_in = nc.dram_tensor("b_h", [H3], dt.float32, kind="ExternalInput").ap()
    h0_in = nc.dram_tensor("h0", [H], dt.float32, kind="ExternalInput").ap()
    mlpw_in = nc.dram_tensor("mlp_W", [N_MLP, H, H], dt.float32,
                             kind="ExternalInput").ap()
    mlpb_in = nc.dram_tensor("mlp_b", [N_MLP, H], dt.float32,
                             kind="ExternalInput").ap()
    lns_in = nc.dram_tensor("ln_scale", [N_MLP, H], dt.float32,
                            kind="ExternalInput").ap()
    lnb_in = nc.dram_tensor("ln_bias", [N_MLP, H], dt.float32,
                            kind="ExternalInput").ap()
    whd_in = nc.dram_tensor("W_heads", [H, NH], dt.float32,
                            kind="ExternalInput").ap()
    bhd_in = nc.dram_tensor("b_heads", [1, NH], dt.float32,
                            kind="ExternalInput").ap()
    bsel_in = nc.dram_tensor("bsel", [P, NCORES], dt.float32,
                             kind="ExternalInput").ap()
    h0sel_in = nc.dram_tensor("h0sel", [P, 1], dt.float32,
                              kind="ExternalInput").ap()

    amps_out = nc.dram_tensor("amps", [TC, N_HARM], dt.float32,
                              kind="ExternalOutput").ap()
    noise_out = nc.dram_tensor("noise", [TC, N_NOISE], dt.float32,
                               kind="ExternalOutput").ap()

    with tile.TileContext(nc) as tc, ExitStack() as ctx:
        dram = ctx.enter_context(tc.tile_pool(name="dram", bufs=1, space="DRAM"))
        gi_dram = dram.tile([M3, P, TC], dt.float32, tag="gi_d")
        hseq_dram = dram.tile([KH, P, TC], dt.float32, tag="hseq_d")
        cc_in = [dram.tile([KH, P], dt.float32, tag=f"ccin{s}",
                           name=f"cc_in{s}") for s in range(NSWEEP - 1)]
        cc_out = [dram.tile([NCORES, KH, P], dt.float32, tag=f"ccout{s}",
                            name=f"cc_out{s}") for s in range(NSWEEP - 1)]

        const = ctx.enter_context(tc.tile_pool(name="const", bufs=1))

        # ---- constants / bias vectors ----
        bh_sb = const.tile([P, M3], dt.float32)
        for m in range(M3):
            nc.sync.dma_start(bh_sb[:, m:m + 1],
                              bh_in.rearrange("(m p) -> m p", p=P)[m, :]
                              .rearrange("(p o) -> p o", o=1))
        bin_sb = const.tile([P, KH], dt.float32)
        for k in range(KH):
            nc.sync.dma_start(bin_sb[:, k:k + 1],
                              bin_in.rearrange("(k p) -> k p", p=P)[k, :]
                              .rearrange("(p o) -> p o", o=1))
        h0_sb = const.tile([P, KH, 1], dt.float32)
        for k in range(KH):
            nc.sync.dma_start(h0_sb[:, k, :],
                              h0_in.rearrange("(k p) -> k p", p=P)[k, :]
                              .rearrange("(p o) -> p o", o=1))
        bsel_sb = const.tile([P, NCORES], dt.float32)
        nc.sync.dma_start(bsel_sb[:], bsel_in[:])
        h0sel_sb = const.tile([P, 1], dt.float32)
        nc.sync.dma_start(h0sel_sb[:], h0sel_in[:])
        mlpb_sb = const.tile([P, N_MLP, KH], dt.float32)
        lns_sb = const.tile([P, N_MLP, KH], dt.float32)
        lnb_sb = const.tile([P, N_MLP, KH], dt.float32)
        for i in range(N_MLP):
            for k in range(KH):
                col = mlpb_in[i, :].rearrange("(k p) -> k p", p=P)[k, :] \
                    .rearrange("(p o) -> p o", o=1)
                nc.sync.dma_start(mlpb_sb[:, i, k:k + 1], col)
                col = lns_in[i, :].rearrange("(k p) -> k p", p=P)[k, :] \
                    .rearrange("(p o) -> p o", o=1)
                nc.sync.dma_start(lns_sb[:, i, k:k + 1], col)
                col = lnb_in[i, :].rearrange("(k p) -> k p", p=P)[k, :] \
                    .rearrange("(p o) -> p o", o=1)
                nc.sync.dma_start(lnb_sb[:, i, k:k + 1], col)
        ones_f = const.tile([P, 2], dt.float32)
        nc.vector.memset(ones_f[:], 1.0)
        ones_r = const.tile([P, 2], F32R)
        nc.vector.tensor_copy(ones_r[:], ones_f[:])
        one_row_f = const.tile([1, P], dt.float32)
        nc.vector.memset(one_row_f[:], 1.0)
        one_row_r = const.tile([1, P], F32R)
        nc.vector.tensor_copy(one_row_r[:], one_row_f[:])
        bhd_rep = const.tile([P, NH], dt.float32)   # replicated head bias
        eps_sb = const.tile([1, 1], dt.float32)
        nc.vector.memset(eps_sb[:], LN_EPS)

        # ============================================================
        # Phase B: feats -> projT -> gi' = Wi^T projT + b_h   (s-order)
        # ============================================================
        with ExitStack() as phb:
            pool = phb.enter_context(tc.tile_pool(name="phb", bufs=2))
            wipool = phb.enter_context(tc.tile_pool(name="wip", bufs=1))
            ppsum = phb.enter_context(tc.tile_pool(name="phb_ps", bufs=3,
                                                   space="PSUM"))

            wi_sb = wipool.tile([P, KH, H3], F32R)
            for k in range(KH):
                wtmp = pool.tile([P, H3], dt.float32, tag="wld")
                nc.sync.dma_start(wtmp[:], wi_in[k * P:(k + 1) * P, :])
                nc.vector.tensor_copy(wi_sb[:, k, :], wtmp[:])

            win_f = wipool.tile([2, H], dt.float32)
            nc.sync.dma_start(win_f[:], win_in[:])
            win_sb = wipool.tile([2, H], F32R)
            nc.vector.tensor_copy(win_sb[:], win_f[:])

            feats_f = wipool.tile([2, TC], dt.float32)
            nc.sync.dma_start(feats_f[0:1, :], f0_in.rearrange("(o t) -> o t", o=1))
            nc.sync.dma_start(feats_f[1:2, :], lo_in.rearrange("(o t) -> o t", o=1))
            feats = wipool.tile([2, TC], F32R)
            nc.vector.tensor_copy(feats[:], feats_f[:])

            # replicated head bias via K=1 ones matmul (built here since
            # phase-B psum pool is alive)
            bhd_f = wipool.tile([1, NH], dt.float32)
            nc.sync.dma_start(bhd_f[:], bhd_in[:])
            bhd_r = wipool.tile([1, NH], F32R)
            nc.vector.tensor_copy(bhd_r[:], bhd_f[:])
            bps = ppsum.tile([P, NH], dt.float32, tag="proj_ps")
            nc.tensor.matmul(bps[:], one_row_r[:], bhd_r[:], start=True,
                             stop=True)
            nc.vector.tensor_copy(bhd_rep[:], bps[:])

            for cch in range(NCH):
                sl = slice(cch * CHUNK, (cch + 1) * CHUNK)
                proj = pool.tile([P, KH, CHUNK], F32R, tag="proj")
                for k in range(KH):
                    pps = ppsum.tile([P, CHUNK], dt.float32, tag="proj_ps")
                    nc.tensor.matmul(pps[:], win_sb[:, k * P:(k + 1) * P],
                                     feats[:, sl], start=True, stop=True)
                    pf = pool.tile([P, CHUNK], dt.float32, tag="projf")
                    nc.scalar.activation(pf[:], pps[:], AF.Identity,
                                         bias=bin_sb[:, k:k + 1])
                    nc.vector.tensor_copy(proj[:, k, :], pf[:])
                for m in range(M3):
                    gps = ppsum.tile([P, CHUNK], dt.float32, tag="gi_ps")
                    for k in range(KH):
                        nc.tensor.matmul(gps[:], wi_sb[:, k, m * P:(m + 1) * P],
                                         proj[:, k, :], start=(k == 0),
                                         stop=(k == KH - 1))
                    gsb = pool.tile([P, CHUNK], dt.float32, tag="gi_sb")
                    nc.scalar.activation(gsb[:], gps[:], AF.Identity,
                                         bias=bh_sb[:, m:m + 1])
                    nc.sync.dma_start(gi_dram[m, :, sl], gsb[:])

        # ============================================================
        # Phase C: block-Picard GRU sweeps
        # ============================================================
        with ExitStack() as phc:
            whpool = phc.enter_context(tc.tile_pool(name="whp", bufs=1))
            wh_sb = whpool.tile([P, KH, H3], F32R)
            spool = phc.enter_context(tc.tile_pool(name="sweep", bufs=2))
            gipool = phc.enter_context(tc.tile_pool(name="gist", bufs=3))
            cpsum = phc.enter_context(tc.tile_pool(name="sw_ps", bufs=6,
                                                   space="PSUM"))
            with ExitStack() as whl:
                wpool_tmp = whl.enter_context(tc.tile_pool(name="whl", bufs=2))
                for k in range(KH):
                    wtmp = wpool_tmp.tile([P, H3], dt.float32, tag="whld")
                    nc.sync.dma_start(wtmp[:], wh_in[k * P:(k + 1) * P, :])
                    nc.vector.tensor_copy(wh_sb[:, k, :], wtmp[:])

            hprev_f = None
            hprev_r = None
            for s in range(NSWEEP):
                hinit_f = spool.tile([P, KH, NB], dt.float32, tag="hstate")
                if s == 0:
                    nc.vector.memset(hinit_f[:], 0.0)
                    nc.vector.scalar_tensor_tensor(
                        hinit_f[:, :, 0:1], h0_sb[:], h0sel_sb[:],
                        hinit_f[:, :, 0:1], ALU.mult, ALU.add)
                else:
                    nc.vector.tensor_copy(hinit_f[:, :, 1:NB],
                                          hprev_f[:, :, 0:NB - 1])
                    nc.vector.memset(hinit_f[:, :, 0:1], 0.0)
                    ag = spool.tile([P, NCORES, KH, 1], dt.float32, tag="ag")
                    for v in range(NCORES):
                        for k in range(KH):
                            nc.sync.dma_start(
                                ag[:, v, k, :],
                                cc_out[s - 1][v, k, :].rearrange("(p o) -> p o", o=1))
                    for v in range(NCORES):
                        nc.vector.scalar_tensor_tensor(
                            hinit_f[:, :, 0:1], ag[:, v, :, :],
                            bsel_sb[:, v:v + 1], hinit_f[:, :, 0:1],
                            ALU.mult, ALU.add)
                    nc.vector.scalar_tensor_tensor(
                        hinit_f[:, :, 0:1], h0_sb[:], h0sel_sb[:],
                        hinit_f[:, :, 0:1], ALU.mult, ALU.add)

                hinit_r = spool.tile([P, KH, NB], F32R, tag="hstater")
                nc.vector.tensor_copy(hinit_r[:], hinit_f[:])
                hprev_f, hprev_r = hinit_f, hinit_r

                final = s == NSWEEP - 1
                for j in range(B):
                    gi_t = gipool.tile([P, M3, NB], dt.float32, tag="gi")
                    nc.sync.dma_start(
                        gi_t[:],
                        gi_dram[:, :, j * NB:(j + 1) * NB]
                        .rearrange("m p n -> p m n"))

                    r_sb = spool.tile([P, KH, NB], dt.float32, tag="r")
                    z_sb = spool.tile([P, KH, NB], dt.float32, tag="z")
                    n_sb = spool.tile([P, KH, NB], dt.float32, tag="n")
                    hnew_f = spool.tile([P, KH, NB], dt.float32, tag="hstate")
                    hnew_r = spool.tile([P, KH, NB], F32R, tag="hstater")

                    for m in range(M3):
                        ps = cpsum.tile([P, NB], dt.float32, tag="gh")
                        for k in range(KH):
                            nc.tensor.matmul(ps[:],
                                             wh_sb[:, k, m * P:(m + 1) * P],
                                             hprev_r[:, k, :], start=(k == 0),
                                             stop=(k == KH - 1))
                        if m < KH:
                            g = m
                            tmp = spool.tile([P, NB], dt.float32, tag="tmp")
                            nc.vector.tensor_tensor(tmp[:], ps[:],
                                                    gi_t[:, m, :], ALU.add)
                            nc.scalar.activation(r_sb[:, g, :], tmp[:],
                                                 AF.Sigmoid)
                        elif m < 2 * KH:
                            g = m - KH
                            tmp = spool.tile([P, NB], dt.float32, tag="tmp")
                            nc.vector.tensor_tensor(tmp[:], ps[:],
                                                    gi_t[:, m, :], ALU.add)
                            nc.scalar.activation(z_sb[:, g, :], tmp[:],
                                                 AF.Sigmoid)
                        else:
                            g = m - 2 * KH
                            tmp = spool.tile([P, NB], dt.float32, tag="tmp")
                            nc.vector.tensor_tensor(tmp[:], ps[:],
                                                    r_sb[:, g, :], ALU.mult)
                            tmp2 = spool.tile([P, NB], dt.float32, tag="tmp2")
                            nc.vector.tensor_tensor(tmp2[:], tmp[:],
                                                    gi_t[:, m, :], ALU.add)
                            nc.scalar.activation(n_sb[:, g, :], tmp2[:],
                                                 AF.Tanh)
                            d = spool.tile([P, NB], dt.float32, tag="d")
                            nc.gpsimd.tensor_tensor(d[:], hprev_f[:, g, :],
                                                    n_sb[:, g, :],
                                                    ALU.subtract)
                            q = spool.tile([P, NB], dt.float32, tag="q")
                            nc.gpsimd.tensor_tensor(q[:], z_sb[:, g, :], d[:],
                                                    ALU.mult)
                            nc.vector.tensor_tensor(hnew_f[:, g, :],
                                                    n_sb[:, g, :], q[:],
                                                    ALU.add)
                    nc.vector.tensor_copy(hnew_r[:], hnew_f[:])
                    if final:
                        nc.sync.dma_start(
                            hseq_dram[:, :, j * NB:(j + 1) * NB]
                            .rearrange("k p n -> p k n"),
                            hnew_f[:])
                    hprev_f, hprev_r = hnew_f, hnew_r

                if s < NSWEEP - 1:
                    for k in range(KH):
                        nc.sync.dma_start(
                            cc_in[s][k, :].rearrange("(p o) -> p o", o=1),
                            hprev_f[:, k, NB - 1:NB])
                    nc.gpsimd.collective_compute(
                        "AllGather", ALU.bypass,
                        replica_groups=[list(range(NCORES))],
                        ins=[cc_in[s][:].opt()],
                        outs=[cc_out[s][:].opt()],
                    )

        # ============================================================
        # Phase D: MLP layers (layer-outer, chunk-inner, DRAM ping-pong)
        #          then heads
        # ============================================================
        mlpA_dram = dram.tile([KH, P, TC], dt.float32, tag="mlpA",
                              name="mlpA_dram")
        mlpB_dram = dram.tile([KH, P, TC], dt.float32, tag="mlpB",
                              name="mlpB_dram")
        layer_io = [(hseq_dram, mlpA_dram), (mlpA_dram, mlpB_dram),
                    (mlpB_dram, mlpA_dram)]

        with ExitStack() as phd:
            wpool = phd.enter_context(tc.tile_pool(name="mlpw", bufs=2))
            mpool = phd.enter_context(tc.tile_pool(name="mlp", bufs=2))
            psA = phd.enter_context(tc.tile_pool(name="ps_a", bufs=3,
                                                 space="PSUM"))
            psB = phd.enter_context(tc.tile_pool(name="ps_b", bufs=1,
                                                 space="PSUM"))

            for layer in range(N_MLP):
                src_d, dst_d = layer_io[layer]
                wl = wpool.tile([P, KH, H], F32R, tag="wl")
                for k in range(KH):
                    wtmp = mpool.tile([P, H], dt.float32, tag="mwld")
                    nc.sync.dma_start(wtmp[:], mlpw_in[layer, k * P:(k + 1) * P, :])
                    nc.vector.tensor_copy(wl[:, k, :], wtmp[:])

                for cch in range(NCH):
                    sl = slice(cch * CHUNK, (cch + 1) * CHUNK)
                    x = mpool.tile([P, KH, CHUNK], F32R, tag="x")
                    for k in range(KH):
                        xf = mpool.tile([P, CHUNK], dt.float32, tag="xf")
                        nc.sync.dma_start(xf[:], src_d[k, :, sl])
                        nc.vector.tensor_copy(x[:, k, :], xf[:])

                    y = mpool.tile([P, KH, CHUNK], dt.float32, tag="y")
                    sums_y = psB.tile([1, CHUNK], dt.float32, tag="sums_y")
                    sums_q = psB.tile([1, CHUNK], dt.float32, tag="sums_q")
                    for m in range(KH):
                        ps = psA.tile([P, CHUNK], dt.float32, tag="mlp_gh")
                        for k in range(KH):
                            nc.tensor.matmul(ps[:],
                                             wl[:, k, m * P:(m + 1) * P],
                                             x[:, k, :], start=(k == 0),
                                             stop=(k == KH - 1))
                        nc.scalar.activation(y[:, m, :], ps[:], AF.Identity,
                                             bias=mlpb_sb[:, layer, m:m + 1])
                        yr = mpool.tile([P, CHUNK], F32R, tag="yr")
                        nc.vector.tensor_copy(yr[:], y[:, m, :])
                        sqf = mpool.tile([P, CHUNK], dt.float32, tag="sqf")
                        nc.scalar.activation(sqf[:], y[:, m, :], AF.Square)
                        sqr = mpool.tile([P, CHUNK], F32R, tag="sqr")
                        nc.vector.tensor_copy(sqr[:], sqf[:])
                        nc.tensor.matmul(sums_y[:], ones_r[:, 0:1], yr[:],
                                         start=(m == 0), stop=(m == KH - 1),
                                         skip_group_check=True)
                        nc.tensor.matmul(sums_q[:], ones_r[:, 1:2], sqr[:],
                                         start=(m == 0), stop=(m == KH - 1),
                                         skip_group_check=True)
                    mu_f = mpool.tile([1, CHUNK], dt.float32, tag="mu_f")
                    nc.vector.tensor_scalar_mul(mu_f[:], sums_y[:], 1.0 / H)
                    mu = mpool.tile([1, CHUNK], F32R, tag="mu")
                    nc.vector.tensor_copy(mu[:], mu_f[:])
                    musq = mpool.tile([1, CHUNK], dt.float32, tag="musq")
                    nc.vector.tensor_tensor(musq[:], mu_f[:], mu_f[:], ALU.mult)
                    var = mpool.tile([1, CHUNK], dt.float32, tag="var")
                    nc.vector.scalar_tensor_tensor(var[:], sums_q[:],
                                                   1.0 / H, musq[:], ALU.mult,
                                                   ALU.subtract)
                    lnv = mpool.tile([1, CHUNK], dt.float32, tag="lnv")
                    nc.scalar.activation(lnv[:], var[:], AF.Ln, bias=eps_sb[:])
                    rstd_f = mpool.tile([1, CHUNK], dt.float32, tag="rstd_f")
                    nc.scalar.activation(rstd_f[:], lnv[:], AF.Exp, scale=-0.5)
                    rstd = mpool.tile([1, CHUNK], F32R, tag="rstd")
                    nc.vector.tensor_copy(rstd[:], rstd_f[:])
                    mub = psB.tile([P, CHUNK], dt.float32, tag="mub")
                    rstdb = psB.tile([P, CHUNK], dt.float32, tag="rstdb")
                    nc.tensor.matmul(mub[:], one_row_r[:], mu[:], start=True,
                                     stop=True)
                    nc.tensor.matmul(rstdb[:], one_row_r[:], rstd[:],
                                     start=True, stop=True)
                    for m in range(KH):
                        tmm = mpool.tile([P, CHUNK], dt.float32, tag="tmm")
                        nc.vector.tensor_tensor(tmm[:], y[:, m, :], mub[:],
                                                ALU.subtract)
                        tm2 = mpool.tile([P, CHUNK], dt.float32, tag="tm2")
                        nc.vector.tensor_tensor(tm2[:], tmm[:], rstdb[:],
                                                ALU.mult)
                        of = mpool.tile([P, CHUNK], dt.float32, tag="of")
                        nc.scalar.activation(of[:], tm2[:], AF.Relu,
                                             bias=lnb_sb[:, layer, m:m + 1],
                                             scale=lns_sb[:, layer, m:m + 1])
                        nc.sync.dma_start(dst_d[m, :, sl], of[:])

        # ---- heads ----
        with ExitStack() as phe:
            hpool = phe.enter_context(tc.tile_pool(name="heads", bufs=2))
            hwpool = phe.enter_context(tc.tile_pool(name="headsw", bufs=1))
            psH = phe.enter_context(tc.tile_pool(name="ps_h", bufs=4,
                                                 space="PSUM"))
            whd_sb = hwpool.tile([P, KH, NH], F32R)
            for k in range(KH):
                wtmp = hpool.tile([P, NH], dt.float32, tag="hwld")
                nc.sync.dma_start(wtmp[:], whd_in[k * P:(k + 1) * P, :])
                nc.vector.tensor_copy(whd_sb[:, k, :], wtmp[:])

            h3_dram = layer_io[N_MLP - 1][1]
            for jj in range(TC // P):
                x = hpool.tile([P, KH, P], F32R, tag="hx")
                for k in range(KH):
                    xf = hpool.tile([P, P], dt.float32, tag="hxf")
                    nc.sync.dma_start(xf[:], h3_dram[k, :, jj * P:(jj + 1) * P])
                    nc.vector.tensor_copy(x[:, k, :], xf[:])
                hps = psH.tile([P, NH], dt.float32, tag="heads")
                for k in range(KH):
                    nc.tensor.matmul(hps[:], x[:, k, :], whd_sb[:, k, :],
                                     start=(k == 0), stop=(k == KH - 1))
                hv = hpool.tile([P, NH], dt.float32, tag="hv")
                nc.vector.tensor_tensor(hv[:], hps[:], bhd_rep[:], ALU.add)
                ex = hpool.tile([P, 1 + N_HARM], dt.float32, tag="ex")
                nc.scalar.activation(ex[:], hv[:, 0:1 + N_HARM], AF.Exp,
                                     scale=-1.0)
                nc.vector.tensor_scalar_add(ex[:], ex[:], 1.0)
                sig = hpool.tile([P, 1 + N_HARM], dt.float32, tag="sig")
                nc.vector.reciprocal(sig[:], ex[:])
                lnt = hpool.tile([P, 1 + N_HARM], dt.float32, tag="lnt")
                nc.scalar.activation(lnt[:], sig[:], AF.Ln)
                es = hpool.tile([P, 1 + N_HARM], dt.float32, tag="es")
                nc.scalar.activation(es[:], lnt[:], AF.Exp, scale=LOG_EXP)
                nc.vector.tensor_scalar(es[:], es[:], 2.0, 1e-7, ALU.mult,
                                        ALU.add)
                hsum = hpool.tile([P, 1], dt.float32, tag="hsum")
                nc.vector.tensor_reduce(hsum[:], es[:, 1:1 + N_HARM],
                                        AX.X, ALU.add)
                nc.vector.tensor_scalar_add(hsum[:], hsum[:], 1e-8)
                rec = hpool.tile([P, 1], dt.float32, tag="rec")
                nc.vector.reciprocal(rec[:], hsum[:])
                fac = hpool.tile([P, 1], dt.float32, tag="fac")
                nc.vector.tensor_tensor(fac[:], es[:, 0:1], rec[:], ALU.mult)
                aout = hpool.tile([P, N_HARM], dt.float32, tag="aout")
                nc.vector.tensor_scalar(aout[:], es[:, 1:1 + N_HARM],
                                        fac[:], None, ALU.mult)
                nc.sync.dma_start(
                    amps_out.rearrange("(b j) h -> b j h", j=B)[:, jj, :],
                    aout[:])
                nout = hpool.tile([P, N_NOISE], dt.float32, tag="nout")
                nc.vector.tensor_copy(nout[:], hv[:, 1 + N_HARM:NH])
                nc.sync.dma_start(
                    noise_out.rearrange("(b j) h -> b j h", j=B)[:, jj, :],
                    nout[:])

    nc.compile()
    return nc


_NC_CACHE = None


def _s_order(x):
    # t = b*B + j  ->  s = j*NB + b
    return np.ascontiguousarray(
        np.asarray(x, np.float32).reshape(NB, B).T.reshape(-1))


def kernel(**inputs):
    global _NC_CACHE
    _install_ntff_hook()

    f0 = np.asarray(inputs["f0"], np.float32)
    loud = np.asarray(inputs["loudness"], np.float32)
    W_heads = np.concatenate(
        [np.asarray(inputs["amp_W"], np.float32),
         np.asarray(inputs["harm_W"], np.float32),
         np.asarray(inputs["noise_W"], np.float32)], axis=1)
    b_heads = np.concatenate(
        [np.asarray(inputs["amp_b"], np.float32),
         np.asarray(inputs["harm_b"], np.float32),
         np.asarray(inputs["noise_b"], np.float32)], axis=0)[None, :]

    shared = {
        "W_in": np.ascontiguousarray(np.asarray(inputs["W_in"], np.float32)),
        "b_in": np.asarray(inputs["b_in"], np.float32),
        "Wi": np.ascontiguousarray(np.asarray(inputs["Wi"], np.float32)),
        "Wh": np.ascontiguousarray(np.asarray(inputs["Wh"], np.float32)),
        "b_h": np.asarray(inputs["b_h"], np.float32),
        "h0": np.asarray(inputs["h0"], np.float32),
        "mlp_W": np.ascontiguousarray(np.asarray(inputs["mlp_W"], np.float32)),
        "mlp_b": np.ascontiguousarray(np.asarray(inputs["mlp_b"], np.float32)),
        "ln_scale": np.ascontiguousarray(np.asarray(inputs["ln_scale"],
                                                    np.float32)),
        "ln_bias": np.ascontiguousarray(np.asarray(inputs["ln_bias"],
                                                   np.float32)),
        "W_heads": np.ascontiguousarray(W_heads),
        "b_heads": np.ascontiguousarray(b_heads),
    }
    in_maps = []
    for c in range(NCORES):
        bsel = np.zeros((P, NCORES), np.float32)
        if c > 0:
            bsel[:, c - 1] = 1.0
        h0sel = np.full((P, 1), 1.0 if c == 0 else 0.0, np.float32)
        in_maps.append({
            **shared,
            "f0c": _s_order(f0[c * TC:(c + 1) * TC]),
            "loudc": _s_order(loud[c * TC:(c + 1) * TC]),
            "bsel": bsel,
            "h0sel": h0sel,
        })

    if _NC_CACHE is None:
        _NC_CACHE = build()
    r = run_bass_kernel_spmd(_NC_CACHE, in_maps, list(range(NCORES)),
                             trace=bool(int(os.environ.get("DDSP_TRACE", "0"))))
    kernel.last_results = r

    amps = np.concatenate([r.results[c]["amps"] for c in range(NCORES)], axis=0)
    noise = np.concatenate([r.results[c]["noise"] for c in range(NCORES)],
                           axis=0)
    return (amps, noise)
